# revision 38
# baseline (speedup 1.0000x reference)
"""GQA (RoPE + causal softmax) Trainium2 Bass kernel, 8-core SPMD.

Sharding: DP over batch (2) x TP over KV groups (4 quarters of heads).
Core c handles batch c//4 and head quarter c%4 (8 q-heads, 2 kv-heads).
Each core computes a partial o_proj ([S, D]); host sums 4 partials per batch.

Matmuls run in float32r (1 col/cyc at N>=256, PE @2.4GHz); attention
probabilities and V are bf16. Everything on-chip is in "transposed"
layout (feature dim on partitions), so x^T is the only host-side prep.

Schedule (PE and Act kept concurrently busy):
 - Phase A: qkv^T = W^T @ x^T per 512-col chunk. Chunk-0 x DMAs are
   interleaved with the weight DMAs so the first matmul starts ~2us in.
   RoPE for chunk n (PE permutation matmul + DVE mul/mul/add) is
   emitted after chunk n+1's projection matmuls so it never stalls the
   PE. PSUM drains alternate DVE/Act.
 - Phase C: v natural tiles [128, 65] bf16 (ones column -> denominator
   rides the AV matmul).
 - Phase D (j-outer): scores for two key blocks go into one [128,1024]
   PSUM tile and get a single wide exp into a persistent bf16 SBUF
   tile. The score->exp chain is Act-paced; the PE's AV matmuls (one
   same-config block per head-pass, run one pass behind) and o_proj
   tiles (one chunk behind) are drawn from a backlog to fill the PE
   during Act waits. Chunk normalization uses reciprocal_approx_fast
   and a select-matmul partition broadcast; attention output reuses
   the qT tiles (their columns are dead after the chunk's scores).
"""

from collections import deque

import numpy as np

import concourse.mybir as mybir
import concourse.tile as tile
from concourse import bacc, bass_utils

B, S, D = 2, 2048, 2048
H, KV, HD = 32, 8, 64
REP = H // KV
SCALE = 1.0 / 8.0  # 1/sqrt(HD)

F32 = mybir.dt.float32
F32R = mybir.dt.float32r
BF16 = mybir.dt.bfloat16
EXP = mybir.ActivationFunctionType.Exp

NCHUNK = S // 512        # 4 sq chunks of 512
NKT = D // 128           # 16 k-tiles over D
NST = S // 128           # 16 sk/s tiles


def _build_program():
    nc = bacc.Bacc()

    xT = nc.dram_tensor("xT", [D, S], BF16, kind="ExternalInput").ap()
    wq = nc.dram_tensor("wq", [D, 8 * HD], BF16, kind="ExternalInput").ap()
    wk = nc.dram_tensor("wk", [D, 2 * HD], BF16, kind="ExternalInput").ap()
    wv = nc.dram_tensor("wv", [D, 2 * HD], BF16, kind="ExternalInput").ap()
    wo = nc.dram_tensor("wo", [8 * HD, D], F32R, kind="ExternalInput").ap()
    cosT2 = nc.dram_tensor("cosT2", [128, S], F32, kind="ExternalInput").ap()
    sinT2 = nc.dram_tensor("sinT2", [128, S], F32, kind="ExternalInput").ap()
    prot = nc.dram_tensor("prot", [128, 128], F32R, kind="ExternalInput").ap()
    tri = nc.dram_tensor("tri", [128, 128], BF16, kind="ExternalInput").ap()
    ident = nc.dram_tensor("ident", [128, 64], F32R, kind="ExternalInput").ap()
    selA = nc.dram_tensor("selA", [128, 512], F32R, kind="ExternalInput").ap()
    selB = nc.dram_tensor("selB", [128, 512], F32R, kind="ExternalInput").ap()
    onescol = nc.dram_tensor("onescol", [128, 1], BF16, kind="ExternalInput").ap()
    zrow = nc.dram_tensor("zrow", [128, 512], F32R, kind="ExternalInput").ap()
    zblk = nc.dram_tensor("zblk", [128, 128], BF16, kind="ExternalInput").ap()
    opart = nc.dram_tensor("opart", [S, D], F32, kind="ExternalOutput").ap()

    with tile.TileContext(nc) as tc:
        with (
            tc.tile_pool(name="persist", bufs=1) as pp,
            tc.tile_pool(name="consts", bufs=1) as cp,
        ):
            # persistent SBUF: q^T/k^T (qT doubles as attn-out storage),
            # denominators, small constants
            qT = [pp.tile([128, S], F32R, tag=f"qT{t}", name=f"qT{t}") for t in range(4)]
            kTz = [pp.tile([128, S], F32R, tag=f"kTz{s}", name=f"kTz{s}") for s in range(2)]

            cosb = cp.tile([128, S], F32, tag="cosb")
            sinb = cp.tile([128, S], F32, tag="sinb")
            protb = cp.tile([128, 128], F32R, tag="protb")
            trib = cp.tile([128, 128], BF16, tag="trib")
            identb = cp.tile([128, 64], F32R, tag="identb")
            selAb = cp.tile([128, 512], F32R, tag="selAb")
            selBb = cp.tile([128, 512], F32R, tag="selBb")
            onesb = cp.tile([128, 1], BF16, tag="onesb")
            zrowb = cp.tile([128, 512], F32R, tag="zrowb")
            zblkb = cp.tile([128, 128], BF16, tag="zblkb")

            vo = [[None] * NST, [None] * NST]
            with tc.tile_pool(name="vop", bufs=1) as vp:  # spans C..D
                with (
                    tc.tile_pool(name="vtbuf", bufs=1) as vtb,
                    tc.tile_pool(name="rotps", bufs=2, space="PSUM") as rpp,
                    tc.tile_pool(name="ropet", bufs=2) as rtp,
                ):
                    vT = vtb.tile([128, S], F32R, tag="vT")

                    def emit_rope(n):
                        ncol = slice(n * 512, (n + 1) * 512)
                        for tl in [*qT, *kTz]:
                            rps = rpp.tile([128, 512], F32, tag="rps")
                            nc.tensor.matmul(rps[:], protb[:], tl[:, ncol],
                                             start=True, stop=True)
                            tmp = rtp.tile([128, 512], F32, tag="ropetmp")
                            nc.vector.tensor_mul(tmp[:], tl[:, ncol], cosb[:, ncol])
                            nc.vector.tensor_mul(rps[:], rps[:], sinb[:, ncol])
                            nc.vector.tensor_add(tl[:, ncol], tmp[:], rps[:])

                    # ---------- Phase A: qkv^T = W^T @ x^T, + RoPE ----------
                    with (
                        tc.tile_pool(name="wts", bufs=1) as wp,
                        tc.tile_pool(name="xin", bufs=16) as xp,
                        tc.tile_pool(name="qkvps", bufs=6, space="PSUM") as pqkv,
                    ):
                        wqk = [wp.tile([128, 8 * HD], BF16, tag=f"wq{k}", name=f"wqk{k}") for k in range(NKT)]
                        wkk = [wp.tile([128, 2 * HD], BF16, tag=f"wk{k}", name=f"wkk{k}") for k in range(NKT)]
                        wvk = [wp.tile([128, 2 * HD], BF16, tag=f"wv{k}", name=f"wvk{k}") for k in range(NKT)]
                        # DMA order: per-k weights interleaved with chunk-0 x
                        # tiles so the first matmuls can start ~2us in.
                        xk0 = []
                        for k in range(NKT):
                            r = slice(k * 128, (k + 1) * 128)
                            nc.sync.dma_start(wqk[k][:], wq[r, :])
                            nc.scalar.dma_start(wkk[k][:], wk[r, :])
                            nc.scalar.dma_start(wvk[k][:], wv[r, :])
                            xk = xp.tile([128, 512], BF16, tag="xk", name=f"xk0_{k}")
                            eng = nc.sync if k % 2 else nc.scalar
                            eng.dma_start(xk[:], xT[r, 0:512])
                            xk0.append(xk)
                        nc.scalar.dma_start(protb[:], prot[:])
                        nc.scalar.dma_start(cosb[:], cosT2[:])
                        nc.scalar.dma_start(sinb[:], sinT2[:])
                        nc.scalar.dma_start(trib[:], tri[:])
                        nc.scalar.dma_start(identb[:], ident[:])
                        nc.scalar.dma_start(selAb[:], selA[:])
                        nc.scalar.dma_start(selBb[:], selB[:])
                        nc.scalar.dma_start(onesb[:], onescol[:])
                        nc.scalar.dma_start(zrowb[:], zrow[:])
                        for c4 in range(NCHUNK):
                            cc = slice(c4 * 512, (c4 + 1) * 512)
                            nc.vector.tensor_copy(kTz[0][64:128, cc], zrowb[64:128, :])
                            nc.vector.tensor_copy(kTz[1][0:64, cc], zrowb[0:64, :])
                        nc.scalar.dma_start(zblkb[:], zblk[:])

                        for n in range(NCHUNK):
                            ncol = slice(n * 512, (n + 1) * 512)
                            accs = [pqkv.tile([128, 512], F32, tag="qkvacc", name=f"acc{n}_{m}") for m in range(6)]
                            for k in range(NKT):
                                if n == 0:
                                    xk = xk0[k]
                                else:
                                    xk = xp.tile([128, 512], BF16, tag="xk")
                                    eng = nc.scalar if k % 2 == 0 else nc.sync
                                    eng.dma_start(xk[:], xT[k * 128:(k + 1) * 128, ncol])
                                st = k == 0
                                sp = k == NKT - 1
                                for t in range(4):
                                    nc.tensor.matmul(
                                        accs[t][:], wqk[k][:, t * 128:(t + 1) * 128],
                                        xk[:], start=st, stop=sp)
                                nc.tensor.matmul(accs[4][:], wkk[k][:], xk[:], start=st, stop=sp)
                                nc.tensor.matmul(accs[5][:], wvk[k][:], xk[:], start=st, stop=sp)
                            # drains alternate DVE / Act so accs free quickly
                            for m, tl in enumerate(qT):
                                if m % 2 == 0:
                                    nc.vector.tensor_copy(tl[:, ncol], accs[m][:])
                                else:
                                    nc.scalar.copy(tl[:, ncol], accs[m][:])
                            nc.vector.tensor_copy(kTz[0][0:64, ncol], accs[4][0:64, :])
                            nc.scalar.copy(kTz[1][64:128, ncol], accs[4][64:128, :])
                            nc.scalar.copy(vT[:, ncol], accs[5][:])
                            # RoPE one chunk behind: its rot matmul depends on
                            # the drain above, so running it inside the next
                            # chunk's matmul stream keeps the PE busy.
                            if n > 0:
                                emit_rope(n - 1)
                            if n == NCHUNK - 1:
                                emit_rope(n)

                    # ---------- Phase C: v natural tiles [128, 65] bf16 ----------
                    with tc.tile_pool(name="vtp", bufs=2, space="PSUM") as vtp:
                        for i in range(NST):
                            for g in range(2):
                                vps = vtp.tile([128, 64], F32R, tag="vps")
                                nc.tensor.transpose(
                                    vps[:], vT[g * 64:(g + 1) * 64, i * 128:(i + 1) * 128],
                                    identb[g * 64:(g + 1) * 64, :])
                                vt = vp.tile([128, 65], BF16, tag=f"vo{g}_{i}", name=f"vo{g}_{i}")
                                nc.vector.tensor_copy(vt[:, 0:64], vps[:])
                                nc.vector.tensor_copy(vt[:, 64:65], onesb[:])
                                vo[g][i] = vt

                # ---------- Phase D: attention + fused normalize/o_proj ----------
                with (
                    tc.tile_pool(name="wop", bufs=1) as wopp,
                    tc.tile_pool(name="esb", bufs=18) as ep,
                    tc.tile_pool(name="rcpp", bufs=2) as rcp_,
                    tc.tile_pool(name="oout", bufs=2) as op,
                    tc.tile_pool(name="sps", bufs=2, space="PSUM") as sp_,
                    tc.tile_pool(name="avp", bufs=2, space="PSUM") as ap_,
                    tc.tile_pool(name="ops", bufs=2, space="PSUM") as opp,
                ):
                    wotp = wopp.tile([128, 4 * D], F32R, tag="wotp")
                    denomA2 = [wopp.tile([128, 512], F32, tag=f"denomA{p}", name=f"denomA{p}") for p in range(2)]
                    denomB2 = [wopp.tile([128, 512], F32, tag=f"denomB{p}", name=f"denomB{p}") for p in range(2)]
                    for p in range(2):
                        nc.gpsimd.memset(denomA2[p][:], 1.0)
                        nc.gpsimd.memset(denomB2[p][:], 1.0)
                    for k4 in range(4):
                        nc.scalar.dma_start(
                            wotp[:, k4 * D:(k4 + 1) * D],
                            wo[k4 * 128:(k4 + 1) * 128, :])

                    backlog = deque()       # AV blocks + drains (es-releasing)
                    backlog2 = deque()      # normalize + o_proj items

                    def drain_backlog(k):
                        while (backlog or backlog2) and k:
                            q = backlog if backlog else backlog2
                            q.popleft()()
                            k -= 1

                    for j in range(NCHUNK):
                        jcol = slice(j * 512, (j + 1) * 512)
                        ni = 4 * j + 4
                        nh = ni // 2
                        denomA = denomA2[j % 2]
                        denomB = denomB2[j % 2]
                        for sub in range(2):
                            for t in range(4):
                                pb = slice(64 * sub, 64 * sub + 64)
                                g = sub
                                avs = ap_.tile([65, 512], F32, tag="avacc",
                                               name=f"av{t}_{j}_{sub}")
                                esl = []
                                # score/exp chain: two key blocks per PSUM
                                # tile, one wide exp each (Act-paced)
                                for h in range(nh):
                                    ssq = sp_.tile([128, 1024], F32, tag="scps")
                                    for s_ in range(2):
                                        i = 2 * h + s_
                                        c0 = max(0, 128 * (i - 4 * j))
                                        ec0 = c0 if 512 - c0 >= 256 else 256
                                        nc.tensor.matmul(
                                            ssq[:, s_ * 512 + ec0:(s_ + 1) * 512],
                                            kTz[sub][:, i * 128:(i + 1) * 128],
                                            qT[t][:, j * 512 + ec0:(j + 1) * 512],
                                            start=True, stop=True)
                                    es = ep.tile([128, 1024], BF16, tag="es")
                                    c0a = max(0, 128 * (2 * h - 4 * j))
                                    nc.scalar.activation(es[:, c0a:1024], ssq[:, c0a:1024],
                                                         EXP, scale=SCALE)
                                    for s_ in range(2):
                                        i = 2 * h + s_
                                        c0 = max(0, 128 * (i - 4 * j))
                                        if i >= 4 * j:
                                            nc.vector.tensor_mul(
                                                es[:, s_ * 512 + c0:s_ * 512 + c0 + 128],
                                                es[:, s_ * 512 + c0:s_ * 512 + c0 + 128],
                                                trib[:])
                                        if c0 == 384:
                                            nc.vector.tensor_copy(
                                                es[:, s_ * 512 + 256:s_ * 512 + 384],
                                                zblkb[:])
                                    esl.append(es)
                                    if h % 2 == 1:
                                        drain_backlog(2)

                                # AV matmuls: one same-config block, run one
                                # pass behind via the backlog
                                def make_av(t=t, sub=sub, g=g, avs=avs, esl=esl,
                                            jv=j, niv=ni, pbv=pb, jc=jcol,
                                            dA=denomA, dB=denomB):
                                    def av_block(i0, i1):
                                        def emit():
                                            for i in range(i0, i1):
                                                c0 = max(0, 128 * (i - 4 * jv))
                                                av0 = c0 if c0 < 384 else 256
                                                s_ = i % 2
                                                es = esl[i // 2]
                                                nc.tensor.matmul(
                                                    avs[:, av0:512], vo[g][i][:],
                                                    es[:, s_ * 512 + av0:(s_ + 1) * 512],
                                                    start=(i == 0), stop=(i == niv - 1))
                                        return emit

                                    def drain():
                                        nc.vector.tensor_copy(qT[t][pbv, jc], avs[0:64, :])
                                        dst = dA if sub == 0 else dB
                                        nc.vector.tensor_copy(
                                            dst[32 * t:32 * t + 1, :], avs[64:65, :])
                                    items = [av_block(i0, min(i0 + 4, niv))
                                             for i0 in range(0, niv, 4)]
                                    items.append(drain)
                                    return items
                                backlog.extend(make_av())

                        # normalize chunk j: backlog items so the next
                        # chunk's score/exp chain overlaps them
                        def make_norm(jv=j, jc=jcol, dA=denomA, dB=denomB):
                            def rcp_item():
                                nc.vector.reciprocal_approx_fast(
                                    out=dA[:], in_=dA[:])
                                nc.vector.reciprocal_approx_fast(
                                    out=dB[:], in_=dB[:])
                                rAB = rcp_.tile([128, 1024], F32R, tag="rAB")
                                nc.vector.tensor_copy(rAB[:, 0:512], dA[:])
                                nc.vector.tensor_copy(rAB[:, 512:1024], dB[:])
                                make_norm.rAB = rAB
                            def bcs_item(t):
                                def emit():
                                    rAB = make_norm.rAB
                                    tsl = slice(t * 128, (t + 1) * 128)
                                    bpsw = sp_.tile([128, 1024], F32, tag="scps")
                                    bps = bpsw[:, 0:512]
                                    nc.tensor.matmul(bps, selAb[:, tsl], rAB[:, 0:512],
                                                     start=True, stop=False)
                                    nc.tensor.matmul(bps, selBb[:, tsl], rAB[:, 512:1024],
                                                     start=False, stop=True)
                                    nc.vector.tensor_mul(qT[t][:, jc], qT[t][:, jc], bps)
                                return emit
                            return [rcp_item] + [bcs_item(t) for t in range(4)]
                        backlog2.extend(make_norm())

                        # o_proj rows for this chunk -> backlog (run during
                        # the next chunk's Act-paced score phase)
                        def make_oproj(jv=j):
                            items = []
                            last = jv == NCHUNK - 1
                            for st in range(4 * jv, 4 * jv + 4):
                                stc = slice(st * 128, (st + 1) * 128)
                                for dch in range(NCHUNK):
                                    def emit(st=st, stc=stc, dch=dch, last=last):
                                        ops = opp.tile([128, 512], F32, tag="opps")
                                        for kt in range(4):
                                            nc.tensor.matmul(
                                                ops[:], qT[kt][:, stc],
                                                wotp[:, kt * D + dch * 512:kt * D + (dch + 1) * 512],
                                                start=(kt == 0), stop=(kt == 3))
                                        oo = op.tile([128, 512], F32, tag="oo")
                                        nc.vector.tensor_copy(oo[:], ops[:])
                                        nc.sync.dma_start(
                                            opart[stc, dch * 512:(dch + 1) * 512],
                                            oo[:])
                                    items.append(emit)
                            return items
                        backlog2.extend(make_oproj())

                    drain_backlog(len(backlog) + len(backlog2))

    nc.compile()
    return nc


_PROGRAM = None


def _get_program():
    global _PROGRAM
    if _PROGRAM is None:
        _PROGRAM = _build_program()
    return _PROGRAM


def _make_in_maps(x, cos, sin, Wq, Wk, Wv, Wo):
    cosT = np.ascontiguousarray(cos.T.astype(np.float32))      # [64, S]
    sinT = np.ascontiguousarray(sin.T.astype(np.float32))
    cosT2 = np.tile(cosT, (2, 1))
    sinT2 = np.tile(sinT, (2, 1))
    import ml_dtypes
    tri = (np.arange(128)[None, :] >= np.arange(128)[:, None]).astype(ml_dtypes.bfloat16)
    ident = np.tile(np.eye(64, dtype=np.float32), (2, 1))
    # rotate-half permutation (sign folded): rot[m] = -tl[m+32] (m%64<32),
    # rot[m] = +tl[m-32] (m%64>=32); out[m,n] = sum_k prot[k,m]*tl[k,n]
    prot = np.zeros((128, 128), dtype=np.float32)
    for m in range(128):
        if m % 64 < 32:
            prot[m + 32, m] = -1.0
        else:
            prot[m - 32, m] = 1.0
    selA = np.zeros((128, 512), dtype=np.float32)
    selB = np.zeros((128, 512), dtype=np.float32)
    for t in range(4):
        selA[32 * t, 128 * t:128 * t + 64] = 1.0
        selB[32 * t, 128 * t + 64:128 * t + 128] = 1.0

    perm = [0, 4, 1, 5, 2, 6, 3, 7]
    in_maps = []
    for c in range(8):
        b, q = c // 4, c % 4
        idx = np.concatenate([np.arange(HD) + (8 * q + j) * HD for j in perm])
        in_maps.append({
            "xT": np.ascontiguousarray(x[b].T.astype(ml_dtypes.bfloat16)),
            "wq": np.ascontiguousarray(Wq[:, idx].astype(ml_dtypes.bfloat16)),
            "wk": np.ascontiguousarray(Wk[:, 2 * q * HD:(2 * q + 2) * HD].astype(ml_dtypes.bfloat16)),
            "wv": np.ascontiguousarray(Wv[:, 2 * q * HD:(2 * q + 2) * HD].astype(ml_dtypes.bfloat16)),
            "wo": np.ascontiguousarray(Wo[idx, :].astype(np.float32)),
            "cosT2": cosT2,
            "sinT2": sinT2,
            "prot": prot,
            "tri": tri,
            "ident": ident,
            "selA": selA,
            "selB": selB,
            "onescol": np.ones((128, 1), dtype=ml_dtypes.bfloat16),
            "zrow": np.zeros((128, 512), dtype=np.float32),
            "zblk": np.zeros((128, 128), dtype=ml_dtypes.bfloat16),
        })
    return in_maps


def _execute(in_maps, trace=False):
    nc = _get_program()
    return bass_utils.run_bass_kernel_spmd(
        nc, in_maps, core_ids=list(range(8)), trace=trace)


def kernel(x, cos, sin, Wq, Wk, Wv, Wo):
    in_maps = _make_in_maps(x, cos, sin, Wq, Wk, Wv, Wo)
    res = _execute(in_maps, trace=False)
    parts = [r["opart"] for r in res.results]
    out = np.empty((B, S, D), dtype=np.float32)
    for b in range(B):
        p = parts[4 * b:4 * b + 4]
        out[b] = (p[0] + p[1]) + (p[2] + p[3])
    return out


# revision 39
# speedup vs baseline: 1.0180x; 1.0180x over previous
"""GQA (RoPE + causal softmax) Trainium2 Bass kernel, 8-core SPMD.

Sharding: DP over batch (2) x TP over KV groups (4 quarters of heads).
Core c handles batch c//4 and head quarter c%4 (8 q-heads, 2 kv-heads).
Each core computes a partial o_proj ([S, D]); host sums 4 partials per batch.

Matmuls run in float32r (1 col/cyc at N>=256, PE @2.4GHz); attention
probabilities and V are bf16. Everything on-chip is in "transposed"
layout (feature dim on partitions), so x^T is the only host-side prep.

Schedule (PE and Act kept concurrently busy):
 - Phase A: qkv^T = W^T @ x^T per 512-col chunk. Chunk-0 x DMAs are
   interleaved with the weight DMAs so the first matmul starts ~2us in.
   RoPE for chunk n (PE permutation matmul + DVE mul/mul/add) is
   emitted after chunk n+1's projection matmuls so it never stalls the
   PE. PSUM drains alternate DVE/Act.
 - Phase C: v natural tiles [128, 65] bf16 (ones column -> denominator
   rides the AV matmul).
 - Phase D (j-outer): scores for two key blocks go into one [128,1024]
   PSUM tile and get a single wide exp into a persistent bf16 SBUF
   tile. The score->exp chain is Act-paced; the PE's AV matmuls (one
   same-config block per head-pass, run one pass behind) and o_proj
   tiles (one chunk behind) are drawn from a backlog to fill the PE
   during Act waits. Chunk normalization uses reciprocal_approx_fast
   and a select-matmul partition broadcast; attention output reuses
   the qT tiles (their columns are dead after the chunk's scores).
"""

from collections import deque

import numpy as np

import concourse.mybir as mybir
import concourse.tile as tile
from concourse import bacc, bass_utils

B, S, D = 2, 2048, 2048
H, KV, HD = 32, 8, 64
REP = H // KV
SCALE = 1.0 / 8.0  # 1/sqrt(HD)

F32 = mybir.dt.float32
F32R = mybir.dt.float32r
BF16 = mybir.dt.bfloat16
EXP = mybir.ActivationFunctionType.Exp

NCHUNK = S // 512        # 4 sq chunks of 512
NKT = D // 128           # 16 k-tiles over D
NST = S // 128           # 16 sk/s tiles


def _build_program():
    nc = bacc.Bacc()

    xT = nc.dram_tensor("xT", [D, S], BF16, kind="ExternalInput").ap()
    wq = nc.dram_tensor("wq", [D, 8 * HD], BF16, kind="ExternalInput").ap()
    wk = nc.dram_tensor("wk", [D, 2 * HD], BF16, kind="ExternalInput").ap()
    wv = nc.dram_tensor("wv", [D, 2 * HD], BF16, kind="ExternalInput").ap()
    wo = nc.dram_tensor("wo", [8 * HD, D], F32R, kind="ExternalInput").ap()
    cosT2 = nc.dram_tensor("cosT2", [128, S], F32, kind="ExternalInput").ap()
    sinT2 = nc.dram_tensor("sinT2", [128, S], F32, kind="ExternalInput").ap()
    prot = nc.dram_tensor("prot", [128, 128], F32R, kind="ExternalInput").ap()
    tri = nc.dram_tensor("tri", [128, 128], BF16, kind="ExternalInput").ap()
    ident = nc.dram_tensor("ident", [128, 64], F32R, kind="ExternalInput").ap()
    selA = nc.dram_tensor("selA", [128, 512], F32R, kind="ExternalInput").ap()
    selB = nc.dram_tensor("selB", [128, 512], F32R, kind="ExternalInput").ap()
    onescol = nc.dram_tensor("onescol", [128, 1], BF16, kind="ExternalInput").ap()
    zrow = nc.dram_tensor("zrow", [128, 512], F32R, kind="ExternalInput").ap()
    zblk = nc.dram_tensor("zblk", [128, 128], BF16, kind="ExternalInput").ap()
    opart = nc.dram_tensor("opart", [S, D], F32, kind="ExternalOutput").ap()

    with tile.TileContext(nc) as tc:
        with (
            tc.tile_pool(name="persist", bufs=1) as pp,
            tc.tile_pool(name="consts", bufs=1) as cp,
        ):
            # persistent SBUF: q^T/k^T (qT doubles as attn-out storage),
            # denominators, small constants
            qT = [pp.tile([128, S], F32R, tag=f"qT{t}", name=f"qT{t}") for t in range(4)]
            kTz = [pp.tile([128, S], F32R, tag=f"kTz{s}", name=f"kTz{s}") for s in range(2)]

            cosb = cp.tile([128, S], F32, tag="cosb")
            sinb = cp.tile([128, S], F32, tag="sinb")
            protb = cp.tile([128, 128], F32R, tag="protb")
            trib = cp.tile([128, 128], BF16, tag="trib")
            identb = cp.tile([128, 64], F32R, tag="identb")
            selAb = cp.tile([128, 512], F32R, tag="selAb")
            selBb = cp.tile([128, 512], F32R, tag="selBb")
            onesb = cp.tile([128, 1], BF16, tag="onesb")
            zrowb = cp.tile([128, 512], F32R, tag="zrowb")
            zblkb = cp.tile([128, 128], BF16, tag="zblkb")

            vo = [[None] * NST, [None] * NST]
            with tc.tile_pool(name="vop", bufs=1) as vp:  # spans C..D
                with (
                    tc.tile_pool(name="vtbuf", bufs=1) as vtb,
                    tc.tile_pool(name="rotps", bufs=2, space="PSUM") as rpp,
                    tc.tile_pool(name="ropet", bufs=2) as rtp,
                ):
                    vT = vtb.tile([128, S], F32R, tag="vT")

                    def emit_rope(n):
                        ncol = slice(n * 512, (n + 1) * 512)
                        for tl in [*qT, *kTz]:
                            rps = rpp.tile([128, 512], F32, tag="rps")
                            nc.tensor.matmul(rps[:], protb[:], tl[:, ncol],
                                             start=True, stop=True)
                            tmp = rtp.tile([128, 512], F32, tag="ropetmp")
                            nc.vector.tensor_mul(tmp[:], tl[:, ncol], cosb[:, ncol])
                            nc.vector.tensor_mul(rps[:], rps[:], sinb[:, ncol])
                            nc.vector.tensor_add(tl[:, ncol], tmp[:], rps[:])

                    # ---------- Phase A: qkv^T = W^T @ x^T, + RoPE ----------
                    with (
                        tc.tile_pool(name="wts", bufs=1) as wp,
                        tc.tile_pool(name="xin", bufs=16) as xp,
                        tc.tile_pool(name="qkvps", bufs=6, space="PSUM") as pqkv,
                    ):
                        wqk = [wp.tile([128, 8 * HD], BF16, tag=f"wq{k}", name=f"wqk{k}") for k in range(NKT)]
                        wkk = [wp.tile([128, 2 * HD], BF16, tag=f"wk{k}", name=f"wkk{k}") for k in range(NKT)]
                        wvk = [wp.tile([128, 2 * HD], BF16, tag=f"wv{k}", name=f"wvk{k}") for k in range(NKT)]
                        # DMA order: per-k weights interleaved with chunk-0 x
                        # tiles so the first matmuls can start ~2us in.
                        xk0 = []
                        for k in range(NKT):
                            r = slice(k * 128, (k + 1) * 128)
                            nc.sync.dma_start(wqk[k][:], wq[r, :])
                            nc.scalar.dma_start(wkk[k][:], wk[r, :])
                            nc.scalar.dma_start(wvk[k][:], wv[r, :])
                            xk = xp.tile([128, 512], BF16, tag="xk", name=f"xk0_{k}")
                            eng = nc.sync if k % 2 else nc.scalar
                            eng.dma_start(xk[:], xT[r, 0:512])
                            xk0.append(xk)
                        nc.scalar.dma_start(protb[:], prot[:])
                        nc.scalar.dma_start(cosb[:], cosT2[:])
                        nc.scalar.dma_start(sinb[:], sinT2[:])
                        nc.scalar.dma_start(trib[:], tri[:])
                        nc.scalar.dma_start(identb[:], ident[:])
                        nc.scalar.dma_start(selAb[:], selA[:])
                        nc.scalar.dma_start(selBb[:], selB[:])
                        nc.scalar.dma_start(onesb[:], onescol[:])
                        nc.scalar.dma_start(zrowb[:], zrow[:])
                        for c4 in range(NCHUNK):
                            cc = slice(c4 * 512, (c4 + 1) * 512)
                            nc.vector.tensor_copy(kTz[0][64:128, cc], zrowb[64:128, :])
                            nc.vector.tensor_copy(kTz[1][0:64, cc], zrowb[0:64, :])
                        nc.scalar.dma_start(zblkb[:], zblk[:])

                        for n in range(NCHUNK):
                            ncol = slice(n * 512, (n + 1) * 512)
                            accs = [pqkv.tile([128, 512], F32, tag="qkvacc", name=f"acc{n}_{m}") for m in range(6)]
                            for k in range(NKT):
                                if n == 0:
                                    xk = xk0[k]
                                else:
                                    xk = xp.tile([128, 512], BF16, tag="xk")
                                    eng = nc.scalar if k % 2 == 0 else nc.sync
                                    eng.dma_start(xk[:], xT[k * 128:(k + 1) * 128, ncol])
                                st = k == 0
                                sp = k == NKT - 1
                                for t in range(4):
                                    nc.tensor.matmul(
                                        accs[t][:], wqk[k][:, t * 128:(t + 1) * 128],
                                        xk[:], start=st, stop=sp)
                                nc.tensor.matmul(accs[4][:], wkk[k][:], xk[:], start=st, stop=sp)
                                nc.tensor.matmul(accs[5][:], wvk[k][:], xk[:], start=st, stop=sp)
                            # drains alternate DVE / Act so accs free quickly
                            for m, tl in enumerate(qT):
                                if m % 2 == 0:
                                    nc.vector.tensor_copy(tl[:, ncol], accs[m][:])
                                else:
                                    nc.scalar.copy(tl[:, ncol], accs[m][:])
                            nc.vector.tensor_copy(kTz[0][0:64, ncol], accs[4][0:64, :])
                            nc.scalar.copy(kTz[1][64:128, ncol], accs[4][64:128, :])
                            nc.scalar.copy(vT[:, ncol], accs[5][:])
                            # RoPE one chunk behind: its rot matmul depends on
                            # the drain above, so running it inside the next
                            # chunk's matmul stream keeps the PE busy.
                            if n > 0:
                                emit_rope(n - 1)

                    # ---------- Phase C: v natural tiles [128, 65] bf16 ----------
                    with tc.tile_pool(name="vtp", bufs=2, space="PSUM") as vtp:
                        for i in range(NST):
                            for g in range(2):
                                vps = vtp.tile([128, 64], F32R, tag="vps")
                                nc.tensor.transpose(
                                    vps[:], vT[g * 64:(g + 1) * 64, i * 128:(i + 1) * 128],
                                    identb[g * 64:(g + 1) * 64, :])
                                vt = vp.tile([128, 65], BF16, tag=f"vo{g}_{i}", name=f"vo{g}_{i}")
                                nc.vector.tensor_copy(vt[:, 0:64], vps[:])
                                nc.vector.tensor_copy(vt[:, 64:65], onesb[:])
                                vo[g][i] = vt
                        emit_rope(NCHUNK - 1)

                # ---------- Phase D: attention + fused normalize/o_proj ----------
                with (
                    tc.tile_pool(name="wop", bufs=1) as wopp,
                    tc.tile_pool(name="esb", bufs=18) as ep,
                    tc.tile_pool(name="rcpp", bufs=2) as rcp_,
                    tc.tile_pool(name="oout", bufs=2) as op,
                    tc.tile_pool(name="sps", bufs=2, space="PSUM") as sp_,
                    tc.tile_pool(name="avp", bufs=2, space="PSUM") as ap_,
                    tc.tile_pool(name="ops", bufs=2, space="PSUM") as opp,
                ):
                    wotp = wopp.tile([128, 4 * D], F32R, tag="wotp")
                    denomA2 = [wopp.tile([128, 512], F32, tag=f"denomA{p}", name=f"denomA{p}") for p in range(2)]
                    denomB2 = [wopp.tile([128, 512], F32, tag=f"denomB{p}", name=f"denomB{p}") for p in range(2)]
                    for p in range(2):
                        nc.gpsimd.memset(denomA2[p][:], 1.0)
                        nc.gpsimd.memset(denomB2[p][:], 1.0)
                    for k4 in range(4):
                        nc.scalar.dma_start(
                            wotp[:, k4 * D:(k4 + 1) * D],
                            wo[k4 * 128:(k4 + 1) * 128, :])

                    backlog = deque()       # AV blocks + drains (es-releasing)
                    backlog2 = deque()      # normalize + o_proj items

                    def drain_backlog(k):
                        while (backlog or backlog2) and k:
                            q = backlog if backlog else backlog2
                            q.popleft()()
                            k -= 1

                    for j in range(NCHUNK):
                        jcol = slice(j * 512, (j + 1) * 512)
                        ni = 4 * j + 4
                        nh = ni // 2
                        denomA = denomA2[j % 2]
                        denomB = denomB2[j % 2]
                        for sub in range(2):
                            for t in range(4):
                                pb = slice(64 * sub, 64 * sub + 64)
                                g = sub
                                avs = ap_.tile([65, 512], F32, tag="avacc",
                                               name=f"av{t}_{j}_{sub}")
                                esl = []
                                # score/exp chain: two key blocks per PSUM
                                # tile, one wide exp each (Act-paced)
                                for h in range(nh):
                                    ssq = sp_.tile([128, 1024], F32, tag="scps")
                                    for s_ in range(2):
                                        i = 2 * h + s_
                                        c0 = max(0, 128 * (i - 4 * j))
                                        ec0 = c0 if 512 - c0 >= 256 else 256
                                        nc.tensor.matmul(
                                            ssq[:, s_ * 512 + ec0:(s_ + 1) * 512],
                                            kTz[sub][:, i * 128:(i + 1) * 128],
                                            qT[t][:, j * 512 + ec0:(j + 1) * 512],
                                            start=True, stop=True)
                                    es = ep.tile([128, 1024], BF16, tag="es")
                                    nc.scalar.activation(es[:], ssq[:], EXP, scale=SCALE)
                                    for s_ in range(2):
                                        i = 2 * h + s_
                                        c0 = max(0, 128 * (i - 4 * j))
                                        if i >= 4 * j:
                                            nc.vector.tensor_mul(
                                                es[:, s_ * 512 + c0:s_ * 512 + c0 + 128],
                                                es[:, s_ * 512 + c0:s_ * 512 + c0 + 128],
                                                trib[:])
                                        if c0 == 384:
                                            nc.vector.tensor_copy(
                                                es[:, s_ * 512 + 256:s_ * 512 + 384],
                                                zblkb[:])
                                    esl.append(es)
                                    if h % 2 == 1:
                                        drain_backlog(2)

                                # AV matmuls: one same-config block, run one
                                # pass behind via the backlog
                                def make_av(t=t, sub=sub, g=g, avs=avs, esl=esl,
                                            jv=j, niv=ni, pbv=pb, jc=jcol,
                                            dA=denomA, dB=denomB):
                                    def av_block(i0, i1):
                                        def emit():
                                            for i in range(i0, i1):
                                                c0 = max(0, 128 * (i - 4 * jv))
                                                av0 = c0 if c0 < 384 else 256
                                                s_ = i % 2
                                                es = esl[i // 2]
                                                nc.tensor.matmul(
                                                    avs[:, av0:512], vo[g][i][:],
                                                    es[:, s_ * 512 + av0:(s_ + 1) * 512],
                                                    start=(i == 0), stop=(i == niv - 1))
                                        return emit

                                    def drain():
                                        nc.vector.tensor_copy(qT[t][pbv, jc], avs[0:64, :])
                                        dst = dA if sub == 0 else dB
                                        nc.vector.tensor_copy(
                                            dst[32 * t:32 * t + 1, :], avs[64:65, :])
                                    items = [av_block(i0, min(i0 + 4, niv))
                                             for i0 in range(0, niv, 4)]
                                    items.append(drain)
                                    return items
                                backlog.extend(make_av())

                        # normalize chunk j: backlog items so the next
                        # chunk's score/exp chain overlaps them
                        def make_norm(jv=j, jc=jcol, dA=denomA, dB=denomB):
                            def rcp_item():
                                nc.vector.reciprocal_approx_fast(
                                    out=dA[:], in_=dA[:])
                                nc.vector.reciprocal_approx_fast(
                                    out=dB[:], in_=dB[:])
                                rAB = rcp_.tile([128, 1024], F32R, tag="rAB")
                                nc.vector.tensor_copy(rAB[:, 0:512], dA[:])
                                nc.vector.tensor_copy(rAB[:, 512:1024], dB[:])
                                make_norm.rAB = rAB
                            def bcs_item(t):
                                def emit():
                                    rAB = make_norm.rAB
                                    tsl = slice(t * 128, (t + 1) * 128)
                                    bpsw = sp_.tile([128, 1024], F32, tag="scps")
                                    bps = bpsw[:, 0:512]
                                    nc.tensor.matmul(bps, selAb[:, tsl], rAB[:, 0:512],
                                                     start=True, stop=False)
                                    nc.tensor.matmul(bps, selBb[:, tsl], rAB[:, 512:1024],
                                                     start=False, stop=True)
                                    nc.vector.tensor_mul(qT[t][:, jc], qT[t][:, jc], bps)
                                return emit
                            return [rcp_item] + [bcs_item(t) for t in range(4)]
                        backlog2.extend(make_norm())

                        # o_proj rows for this chunk -> backlog (run during
                        # the next chunk's Act-paced score phase)
                        def make_oproj(jv=j):
                            items = []
                            last = jv == NCHUNK - 1
                            for st in range(4 * jv, 4 * jv + 4):
                                stc = slice(st * 128, (st + 1) * 128)
                                for dch in range(NCHUNK):
                                    def emit(st=st, stc=stc, dch=dch, last=last):
                                        ops = opp.tile([128, 512], F32, tag="opps")
                                        for kt in range(4):
                                            nc.tensor.matmul(
                                                ops[:], qT[kt][:, stc],
                                                wotp[:, kt * D + dch * 512:kt * D + (dch + 1) * 512],
                                                start=(kt == 0), stop=(kt == 3))
                                        oo = op.tile([128, 512], F32, tag="oo")
                                        nc.vector.tensor_copy(oo[:], ops[:])
                                        nc.sync.dma_start(
                                            opart[stc, dch * 512:(dch + 1) * 512],
                                            oo[:])
                                    items.append(emit)
                            return items
                        backlog2.extend(make_oproj())

                    drain_backlog(len(backlog) + len(backlog2))

    nc.compile()
    return nc


_PROGRAM = None


def _get_program():
    global _PROGRAM
    if _PROGRAM is None:
        _PROGRAM = _build_program()
    return _PROGRAM


def _make_in_maps(x, cos, sin, Wq, Wk, Wv, Wo):
    cosT = np.ascontiguousarray(cos.T.astype(np.float32))      # [64, S]
    sinT = np.ascontiguousarray(sin.T.astype(np.float32))
    cosT2 = np.tile(cosT, (2, 1))
    sinT2 = np.tile(sinT, (2, 1))
    import ml_dtypes
    tri = (np.arange(128)[None, :] >= np.arange(128)[:, None]).astype(ml_dtypes.bfloat16)
    ident = np.tile(np.eye(64, dtype=np.float32), (2, 1))
    # rotate-half permutation (sign folded): rot[m] = -tl[m+32] (m%64<32),
    # rot[m] = +tl[m-32] (m%64>=32); out[m,n] = sum_k prot[k,m]*tl[k,n]
    prot = np.zeros((128, 128), dtype=np.float32)
    for m in range(128):
        if m % 64 < 32:
            prot[m + 32, m] = -1.0
        else:
            prot[m - 32, m] = 1.0
    selA = np.zeros((128, 512), dtype=np.float32)
    selB = np.zeros((128, 512), dtype=np.float32)
    for t in range(4):
        selA[32 * t, 128 * t:128 * t + 64] = 1.0
        selB[32 * t, 128 * t + 64:128 * t + 128] = 1.0

    perm = [0, 4, 1, 5, 2, 6, 3, 7]
    in_maps = []
    for c in range(8):
        b, q = c // 4, c % 4
        idx = np.concatenate([np.arange(HD) + (8 * q + j) * HD for j in perm])
        in_maps.append({
            "xT": np.ascontiguousarray(x[b].T.astype(ml_dtypes.bfloat16)),
            "wq": np.ascontiguousarray(Wq[:, idx].astype(ml_dtypes.bfloat16)),
            "wk": np.ascontiguousarray(Wk[:, 2 * q * HD:(2 * q + 2) * HD].astype(ml_dtypes.bfloat16)),
            "wv": np.ascontiguousarray(Wv[:, 2 * q * HD:(2 * q + 2) * HD].astype(ml_dtypes.bfloat16)),
            "wo": np.ascontiguousarray(Wo[idx, :].astype(np.float32)),
            "cosT2": cosT2,
            "sinT2": sinT2,
            "prot": prot,
            "tri": tri,
            "ident": ident,
            "selA": selA,
            "selB": selB,
            "onescol": np.ones((128, 1), dtype=ml_dtypes.bfloat16),
            "zrow": np.zeros((128, 512), dtype=np.float32),
            "zblk": np.zeros((128, 128), dtype=ml_dtypes.bfloat16),
        })
    return in_maps


def _execute(in_maps, trace=False):
    nc = _get_program()
    return bass_utils.run_bass_kernel_spmd(
        nc, in_maps, core_ids=list(range(8)), trace=trace)


def kernel(x, cos, sin, Wq, Wk, Wv, Wo):
    in_maps = _make_in_maps(x, cos, sin, Wq, Wk, Wv, Wo)
    res = _execute(in_maps, trace=False)
    parts = [r["opart"] for r in res.results]
    out = np.empty((B, S, D), dtype=np.float32)
    for b in range(B):
        p = parts[4 * b:4 * b + 4]
        out[b] = (p[0] + p[1]) + (p[2] + p[3])
    return out
